# revision 12
# baseline (speedup 1.0000x reference)
"""AttnBlock (GroupNorm -> QKV 1x1 -> full NxN attention -> proj -> residual)
for Trainium2, SPMD over 8 NeuronCores.  v2.

Sharding: data-parallel over batch (2) x query-pixel blocks (4 of 1024 px).
Each core gets its batch image x [C, N] and its query slice xq [C, NQ];
K and V^T are computed redundantly per batch pair, queries are disjoint.
No collectives.

Key differences vs v1:
  - x and xq are DMA'd ONCE and stay resident in SBUF (f32).  GroupNorm is
    folded into the QKV weights instead of materializing hn:
        k = Wk^T hn + bk,  hn = A*x + B (per-channel affine from GN stats)
          = (Wk*A)^T x + (Wk^T B + bk)
    After the bn_stats/bn_aggr pass the weights are scaled by A on DVE and
    the extra bias Wk^T B is computed with tiny [128,1]-output matmuls
    directly in [P, CS] layout.  V's bias shift commutes through softmax
    (weights sum to 1) and is folded into the proj bias on device:
        extra_out = Wp^T (Wv^T B).
  - Weights / K / V^T / Q are bf16 (PE: 1 cycle/row, half SBUF+DMA);
    x stays f32 and is bitcast to f32r as a matmul operand (1 cycle/row
    for free dim >= 256).  Accumulation is always fp32 in PSUM; exp tiles
    are f32r so softmax denominators accumulate in full precision on DVE.
  - All PSUM->SBUF drains go through ACT (Identity+bias) to keep DVE free.

HBM traffic per core: x 8MB + xq 2MB + weights 2MB (bf16) + out 2MB = 14MB.
PE floor: K/V proj 2x27us + Q/proj 2x7us + S/AV 2x55us + transposes ~ 184us.
"""

from contextlib import ExitStack

import numpy as np

import concourse.bacc as bacc
import concourse.bass as bass
import concourse.mybir as mybir
import concourse.tile as tile

F32 = mybir.dt.float32
F32R = mybir.dt.float32r
BF16 = mybir.dt.bfloat16
AF = mybir.ActivationFunctionType
MULT = mybir.AluOpType.mult


def build_program(C=512, G=32, N=4096, NQ=1024, eps=1e-5, precision="bf16"):
    """Emit the per-core Bass program (SPMD; per-core data differs only)."""
    P = 128
    CS = C // P                  # channel subtiles (4)
    KT = N // P                  # key/pixel tiles (32)
    NCH = min(512, N)            # x chunk width (pixels) for DMA/stats
    NCHUNKS = N // NCH           # 8
    QP = min(512, NQ)            # query-pass width
    QPASSES = NQ // QP           # 2
    QS = QP // P                 # query subtiles per pass (4)
    cpg = C // G                 # channels per group (16)
    GPS = P // cpg               # groups per partition block (8)
    assert C % P == 0 and N % NCH == 0 and NQ % QP == 0 and P % cpg == 0

    nc = bacc.Bacc(None, target_bir_lowering=False)

    # inputs are packed into TWO tensors to minimize per-dispatch buffers:
    #   xpack 1-D f32 : [x (C*N) | xq (C*NQ) | cpack (P*(8+5*CS)) | inde]
    #     where cpack = [indg | gammaT | betaT | bqT | bkT | bpT]
    #   wall  [C, 4C] bf16 : [wkt | wvt | wqt | wpt] (already GN-transposed)
    NCP = GPS + 5 * CS
    off_xq = C * N
    off_cp = off_xq + C * NQ
    off_in = off_cp + P * NCP
    xpack_d = nc.dram_tensor("xpack", [off_in + GPS * P], F32,
                             kind="ExternalInput")
    wall_d = nc.dram_tensor("wall", [C, 4 * C], BF16, kind="ExternalInput")
    out_d = nc.dram_tensor("out", [C, NQ], F32, kind="ExternalOutput")

    x_r = xpack_d[0:off_xq].rearrange("(s p n) -> p s n", p=P, n=N)
    xq_r = xpack_d[off_xq:off_cp].rearrange("(s p n) -> p s n", p=P, n=NQ)
    cpack_r = xpack_d[off_cp:off_in].rearrange("(p c) -> p c", c=NCP)
    inde_r = xpack_d[off_in:off_in + GPS * P].rearrange("(g p) -> g p", p=P)
    w_r = {
        w: wall_d[:, i * C:(i + 1) * C].rearrange("(s p) c -> p s c", p=P)
        for i, w in enumerate(("k", "v", "q", "p"))
    }
    out_r = out_d[:, :].rearrange("(s p) n -> p s n", p=P)

    with tile.TileContext(nc) as tc, ExitStack() as st:
        const = st.enter_context(tc.tile_pool(name="const", bufs=1))
        big = st.enter_context(tc.tile_pool(name="big", bufs=1))
        small = st.enter_context(tc.tile_pool(name="small", bufs=1))
        ps_sh = st.enter_context(tc.tile_pool(name="ps_sh", bufs=3, space="PSUM"))
        ps_o = st.enter_context(tc.tile_pool(name="ps_o", bufs=CS, space="PSUM"))
        ps_sum = st.enter_context(tc.tile_pool(name="ps_sum", bufs=1, space="PSUM"))

        # ---- constants / host params (scalar queue, before the big x) -----
        cpack = const.tile([P, NCP], F32, tag="cpack")
        nc.scalar.dma_start(out=cpack, in_=cpack_r)
        indg = cpack[:, 0:GPS]
        gammaT = cpack[:, GPS:GPS + CS]
        betaT = cpack[:, GPS + CS:GPS + 2 * CS]
        bT_host = {
            name: cpack[:, GPS + (2 + i) * CS:GPS + (3 + i) * CS]
            for i, name in enumerate(("q", "k", "p"))
        }
        inde = const.tile([GPS, P], F32, tag="inde")
        nc.scalar.dma_start(out=inde, in_=inde_r)
        ones_r = const.tile([P, 1], F32, tag="ones_r")
        nc.vector.memset(ones_r, 1.0)
        ones_row_f = const.tile([1, P], F32, tag="ones_row_f")
        nc.vector.memset(ones_row_f, 1.0)
        # compute-produced f32r copy: the BIR verifier requires f32r matmul
        # operands to be rounded by a compute op, not memset/bitcast
        ones_row = const.tile([1, P], F32R, tag="ones_row")
        nc.vector.tensor_copy(out=ones_row, in_=ones_row_f)
        eps_t = const.tile([P, 1], F32, tag="eps")
        nc.vector.memset(eps_t, eps)

        # ---- resident tensors ---------------------------------------------
        # x lives on in bf16 only (matmul operand); the f32 chunks are
        # consumed for exact GN stats while in a small landing buffer.
        # xq keeps an f32 copy for the exact residual add.
        x_bf = big.tile([P, CS, N], BF16, tag="x")       # 32 KiB/part
        xq_sb = big.tile([P, CS, NQ], F32, tag="xq")     # 16 KiB/part
        xq_bf = big.tile([P, CS, NQ], BF16, tag="xqb")   # 8 KiB/part
        K_sb = big.tile([P, CS, N], BF16, tag="K")       # 16 KiB/part
        VT_sb = big.tile([P, KT, C], BF16, tag="VT")     # 16 KiB/part
        Q_sb = big.tile([P, CS, NQ], BF16, tag="Q")      # 4 KiB/part
        # raw q/k/v weights die after phase 1c -> scoped pool, closed below
        st_w = ExitStack()
        wtmp = st_w.enter_context(tc.tile_pool(name="wtmp", bufs=1))
        w_raw = {"p": big.tile([P, CS, C], BF16, tag="wr_p", name="wr_p")}
        for w in ("q", "k", "v"):
            w_raw[w] = wtmp.tile([P, CS, C], BF16, tag=f"wr_{w}",
                                 name=f"wr_{w}")         # 4 KiB/part each
        w_scl = {w: big.tile([P, CS, C], BF16, tag=f"ws_{w}", name=f"ws_{w}")
                 for w in ("q", "k", "v")}               # A-scaled weights

        # ---- phase 0: DMA in; bn_stats per chunk as it lands --------------
        # x streams first (it gates the GN stats); weights/xq follow.
        # All big DMAs go through one HWDGE queue (sync) in priority order:
        # x chunks (gate the GN stats), then weights, then xq.  gpsimd DMA
        # would go through the slow SWDGE path and get serviced late.
        stats_all = small.tile([P, CS, NCHUNKS, 6], F32, tag="stats")
        st_x = ExitStack()
        xch_pool = st_x.enter_context(tc.tile_pool(name="xch", bufs=4))
        for ch in range(NCHUNKS):
            xc = xch_pool.tile([P, CS, NCH], F32, tag="xc")
            nc.sync.dma_start(out=xc, in_=x_r[:, :, ch * NCH:(ch + 1) * NCH])
            for s in range(CS):
                nc.vector.bn_stats(out=stats_all[:, s, ch, :],
                                   in_=xc[:, s, :])
            # bf16 cast on ACT so the DVE stats chain is never blocked
            nc.scalar.activation(
                out=x_bf[:, :, ch * NCH:(ch + 1) * NCH], in_=xc,
                func=AF.Identity, scale=1.0)
        for w in ("k", "v", "q", "p"):
            nc.sync.dma_start(out=w_raw[w], in_=w_r[w])
        nc.sync.dma_start(out=xq_sb, in_=xq_r)

        # ---- phase 1: GN stats -> per-channel A (scale), B (shift) --------
        mv = small.tile([P, CS, 2], F32, tag="mv")
        for s in range(CS):
            nc.vector.bn_aggr(out=mv[:, s, :], in_=stats_all[:, s, :, :])
        # per-channel mean / E[x^2] -> group reduce via indicator matmul
        rhs8 = small.tile([P, 2 * CS], F32, tag="rhs8")
        nc.vector.tensor_copy(out=rhs8[:, 0:CS], in_=mv[:, :, 0])
        nc.vector.tensor_mul(out=rhs8[:, CS:], in0=mv[:, :, 0], in1=mv[:, :, 0])
        nc.vector.tensor_add(out=rhs8[:, CS:], in0=rhs8[:, CS:], in1=mv[:, :, 1])
        # indg is pre-scaled by 1/cpg on the host, so ps_g holds the group
        # means of [mean | E[x^2]] directly.
        ps_g = ps_sh.tile([GPS, 2 * CS], F32, tag="sbank")
        nc.tensor.matmul(ps_g, lhsT=indg, rhs=rhs8, start=True, stop=True)
        # gvar = E[x^2] - mean^2 ; grstd = 1/sqrt(gvar + eps).  gmean is
        # copied to SBUF on ACT first (DVE has a single PSUM read port, so
        # the square must read the SBUF copy).
        e8 = small.tile([GPS, 2 * CS], F32, tag="e8")
        nc.vector.tensor_copy(out=e8[:, CS:], in_=ps_g[:, 0:CS])
        gsq = small.tile([GPS, CS], F32, tag="gsq")
        nc.vector.tensor_mul(out=gsq, in0=e8[:, CS:], in1=e8[:, CS:])
        nc.vector.tensor_sub(out=e8[:, 0:CS], in0=ps_g[:, CS:], in1=gsq)
        nc.scalar.activation(out=e8[:, 0:CS], in_=e8[:, 0:CS], func=AF.Sqrt,
                             bias=eps_t[:GPS], scale=1.0)
        nc.vector.reciprocal(out=e8[:, 0:CS], in_=e8[:, 0:CS])
        # expand groups -> channels
        ps_e = ps_sh.tile([P, 2 * CS], F32, tag="sbank")
        nc.tensor.matmul(ps_e, lhsT=inde, rhs=e8, start=True, stop=True)
        A_sb = small.tile([P, CS], F32, tag="A")
        nc.vector.tensor_mul(out=A_sb, in0=ps_e[:, 0:CS], in1=gammaT)
        B_sb = small.tile([P, CS], F32, tag="B")
        nc.vector.tensor_mul(out=B_sb, in0=ps_e[:, CS:], in1=A_sb)
        nc.vector.tensor_sub(out=B_sb, in0=betaT, in1=B_sb)
        B_bf = small.tile([P, CS], BF16, tag="Bbf")
        nc.vector.tensor_copy(out=B_bf, in_=B_sb)

        # ---- phase 1b: bias vectors W^T B via tiny matmuls ----------------
        # out column os holds (W[:, os*128:(os+1)*128])^T @ B, i.e. the extra
        # bias for output channels os*128..os*128+127, already in [P, CS]
        # layout for ACT's per-partition bias argument.
        def wtb(wname):
            ps_b = ps_sh.tile([P, CS], F32, tag="sbank", name=f"ps_b_{wname}")
            for os_ in range(CS):
                for s in range(CS):
                    nc.tensor.matmul(
                        ps_b[:, os_:os_ + 1],
                        lhsT=w_raw[wname][:, s, os_ * P:(os_ + 1) * P],
                        rhs=B_bf[:, s:s + 1],
                        start=(s == 0), stop=(s == CS - 1),
                        skip_group_check=True,
                    )
            return ps_b

        # ---- phase 1c (k only) first: wk-scale alone gates the first
        # phase-2 matmuls; wv/wq scaling is deferred below the bias block so
        # neither the DVE chain nor the PE stream blocks the K start
        def scale_w(w):
            for s in range(CS):
                nc.vector.tensor_scalar(
                    w_scl[w][:, s, :], w_raw[w][:, s, :],
                    scalar1=A_sb[:, s:s + 1], scalar2=None, op0=MULT,
                )

        scale_w("k")

        bKT = small.tile([P, CS], F32, tag="bKT")
        nc.vector.tensor_add(out=bKT, in0=wtb("k"), in1=bT_host["k"])
        bQT = small.tile([P, CS], F32, tag="bQT")
        nc.vector.tensor_add(out=bQT, in0=wtb("q"), in1=bT_host["q"])
        # V's shift commutes through softmax; fold through proj:
        # extra_out = Wp^T (Wv^T B)
        bV_bf = small.tile([P, CS], BF16, tag="bVbf")
        nc.vector.tensor_copy(out=bV_bf, in_=wtb("v"))
        ps_pb = ps_sh.tile([P, CS], F32, tag="sbank", name="ps_pb")
        for js in range(CS):
            for os_ in range(CS):
                nc.tensor.matmul(
                    ps_pb[:, js:js + 1],
                    lhsT=w_raw["p"][:, os_, js * P:(js + 1) * P],
                    rhs=bV_bf[:, os_:os_ + 1],
                    start=(os_ == 0), stop=(os_ == CS - 1),
                    skip_group_check=True,
                )
        bPT = small.tile([P, CS], F32, tag="bPT")
        nc.vector.tensor_add(out=bPT, in0=ps_pb, in1=bT_host["p"])

        scale_w("v")
        scale_w("q")

        st_x.close()   # free the f32 chunk landing buffers (LIFO order)
        st_w.close()   # free raw q/k/v weight SBUF for phase-3 pools

        # ---- phase 2: K, V^T, Q from resident bf16 x ----------------------
        x_f = x_bf
        xq_f = xq_bf
        for ch in range(NCHUNKS):
            lo, hi = ch * NCH, (ch + 1) * NCH
            for cs in range(CS):                  # K rows [co-sub, chunk]
                ps_k = ps_sh.tile([P, NCH], F32, tag="sbank")
                for s in range(CS):
                    nc.tensor.matmul(
                        ps_k, lhsT=w_scl["k"][:, s, cs * P:(cs + 1) * P],
                        rhs=x_f[:, s, lo:hi],
                        start=(s == 0), stop=(s == CS - 1),
                    )
                nc.scalar.activation(
                    out=K_sb[:, cs, lo:hi], in_=ps_k,
                    func=AF.Identity, bias=bKT[:, cs:cs + 1], scale=1.0,
                )
            for ns in range(NCH // P):            # V^T rows [pixel-sub, co]
                ps_v = ps_sh.tile([P, C], F32, tag="sbank")
                for s in range(CS):
                    nc.tensor.matmul(
                        ps_v, lhsT=x_f[:, s, lo + ns * P:lo + (ns + 1) * P],
                        rhs=w_scl["v"][:, s, :],
                        start=(s == 0), stop=(s == CS - 1),
                    )
                nc.scalar.activation(
                    out=VT_sb[:, ch * (NCH // P) + ns, :], in_=ps_v,
                    func=AF.Identity, scale=1.0,
                )
        nc.vector.tensor_copy(out=xq_bf, in_=xq_sb)   # DVE is idle here
        for qc in range(NQ // NCH):               # Q rows (own block only)
            lo, hi = qc * NCH, (qc + 1) * NCH
            for cs in range(CS):
                ps_q = ps_sh.tile([P, NCH], F32, tag="sbank")
                for s in range(CS):
                    nc.tensor.matmul(
                        ps_q, lhsT=w_scl["q"][:, s, cs * P:(cs + 1) * P],
                        rhs=xq_f[:, s, lo:hi],
                        start=(s == 0), stop=(s == CS - 1),
                    )
                nc.scalar.activation(
                    out=Q_sb[:, cs, lo:hi], in_=ps_q,
                    func=AF.Identity, bias=bQT[:, cs:cs + 1], scale=1.0,
                )
        # pre-add the proj bias into the f32 residual (DVE idle here), so the
        # output tail is a single DVE add (PSUM proj + biased residual)
        for cs in range(CS):
            nc.vector.tensor_scalar(
                xq_sb[:, cs, :], xq_sb[:, cs, :],
                scalar1=bPT[:, cs:cs + 1], scalar2=None,
                op0=mybir.AluOpType.add,
            )

        # ---- phase 3: attention + proj + residual, per query pass ---------
        # AV is computed directly in [c, q] orientation (lhsT = V^T tile,
        # rhs = exp tile), so no PE transposes are needed before proj.  The
        # softmax denominator row is folded with one ones^T matmul, inverted
        # on DVE, broadcast to all partitions with a K=1 matmul, and applied
        # during the fused PSUM->SBUF normalize+bf16-cast on DVE.
        # Each pass's tail (denominator fold / normalize / proj / out) is
        # interleaved into the NEXT pass's S/AV stream so the PE never idles
        # at a pass boundary.
        with ExitStack() as st2:
            ptp = st2.enter_context(tc.tile_pool(name="ptp", bufs=4))
            ocq = st2.enter_context(tc.tile_pool(name="ocq", bufs=2))
            outp = st2.enter_context(tc.tile_pool(name="outp", bufs=4))
            sm2 = st2.enter_context(tc.tile_pool(name="sm2", bufs=2))

            def make_tail(qp, q0, acc, oc_ps, c0=0, c1=QP):
                """Tail stages for query columns [c0, c1) of pass qp."""
                cw = c1 - c0
                state = {}

                def st_sums():
                    sums = ps_sum.tile([P, QP], F32, tag="sums",
                                       name=f"sums_{qp}_{c0}")
                    nc.tensor.matmul(sums[0:1, c0:c1], lhsT=ones_r,
                                     rhs=acc[:, c0:c1], start=True, stop=True)
                    state["sums"] = sums

                def st_norm():
                    rec_row = sm2.tile([1, QP], F32R, tag="rec_row",
                                       name=f"rec_{qp}_{c0}")
                    with nc.allow_low_precision(
                            reason="f32r operand for broadcast matmul"):
                        nc.vector.reciprocal(out=rec_row[:, 0:cw],
                                             in_=state["sums"][0:1, c0:c1])
                    rec_bc = ps_sum.tile([P, QP], F32, tag="sums",
                                         name=f"rbc_{qp}_{c0}")
                    nc.tensor.matmul(rec_bc[:, 0:cw], lhsT=ones_row,
                                     rhs=rec_row[:, 0:cw],
                                     start=True, stop=True)
                    rb = sm2.tile([P, QP], F32, tag="rb",
                                  name=f"rb_{qp}_{c0}")
                    nc.scalar.activation(out=rb[:, 0:cw],
                                         in_=rec_bc[:, 0:cw],
                                         func=AF.Identity, scale=1.0)
                    oc = state["oc"] = ocq.tile([P, CS, QP], BF16, tag="ocq",
                                                name=f"oc_{qp}_{c0}")
                    with nc.allow_low_precision(
                            reason="bf16 proj operand; fp32 psum accum"):
                        for cs in range(CS):
                            nc.vector.tensor_mul(out=oc[:, cs, c0:c1],
                                                 in0=oc_ps[cs][:, c0:c1],
                                                 in1=rb[:, 0:cw])

                def st_proj(cs):
                    oc = state["oc"]
                    ps_p = ps_sh.tile([P, QP], F32, tag="sbank",
                                      name=f"ps_p_{qp}_{cs}_{c0}")
                    for s in range(CS):
                        nc.tensor.matmul(
                            ps_p[:, 0:cw],
                            lhsT=w_raw["p"][:, s, cs * P:(cs + 1) * P],
                            rhs=oc[:, s, c0:c1],
                            start=(s == 0), stop=(s == CS - 1),
                        )
                    ot = outp.tile([P, QP], F32, tag="ot",
                                   name=f"ot_{qp}_{cs}_{c0}")
                    # proj bias was pre-added into xq_sb: one DVE add drains
                    # PSUM, applies bias + residual in a single hop
                    nc.vector.tensor_add(out=ot[:, 0:cw], in0=ps_p[:, 0:cw],
                                         in1=xq_sb[:, cs, q0 + c0:q0 + c1])
                    nc.sync.dma_start(out=out_r[:, cs, q0 + c0:q0 + c1],
                                      in_=ot[:, 0:cw])

                return [st_sums, st_norm] + [
                    (lambda cs=cs: st_proj(cs)) for cs in range(CS)]

            tail = []
            for qp in range(QPASSES):
                q0 = qp * QP
                oc_ps = [ps_o.tile([P, QP], F32, tag="o", name=f"o_{qp}_{cs}")
                         for cs in range(CS)]
                # softmax denominators: running DVE accumulator over the exp
                # tiles (k stays on partitions), folded in the tail.
                acc = sm2.tile([P, QP], F32, tag="acc")
                # software-pipelined: emit S(kt+1) before AV(kt) so the PE
                # has dense work while ACT computes exp(kt).
                pt_q = []

                def emit_s(kt, qp=qp, q0=q0, pt_q=pt_q):
                    s_ps = ps_sh.tile([P, QP], F32, tag="sbank",
                                      name=f"s_ps_{qp}_{kt}")
                    for s in range(CS):
                        nc.tensor.matmul(
                            s_ps, lhsT=K_sb[:, s, kt * P:(kt + 1) * P],
                            rhs=Q_sb[:, s, q0:q0 + QP],
                            start=(s == 0), stop=(s == CS - 1),
                        )
                    pt = ptp.tile([P, QP], BF16, tag="pt",
                                  name=f"pt_{qp}_{kt}")
                    nc.scalar.activation(out=pt, in_=s_ps, func=AF.Exp)
                    pt_q.append((kt, pt))

                emit_s(0)
                if tail:
                    tail.pop(0)()        # prev pass: denominator fold
                emit_s(1)
                if tail:
                    tail.pop(0)()        # prev pass: normalize (frees o_ps)
                emit_s(2)
                for kt in range(KT):
                    if kt + 3 < KT:
                        emit_s(kt + 3)
                    k0, pt = pt_q.pop(0)
                    assert k0 == kt
                    if kt == 0:
                        nc.vector.tensor_copy(out=acc, in_=pt)
                    else:
                        nc.vector.tensor_add(out=acc, in0=acc, in1=pt)
                    last = kt == KT - 1
                    for cs in range(CS):
                        nc.tensor.matmul(
                            oc_ps[cs], lhsT=VT_sb[:, kt, cs * P:(cs + 1) * P],
                            rhs=pt,
                            start=(kt == 0), stop=last,
                        )
                    if tail and 2 <= kt <= CS + 1:
                        tail.pop(0)()    # prev pass: proj cs = kt-1
                assert not tail
                tail = make_tail(qp, q0, acc, oc_ps)
            for st_fn in tail:
                st_fn()

    nc.finalize()
    return nc


def make_consts(P=128, cpg=16):
    GPS = P // cpg
    indg = np.zeros((P, GPS), np.float32)
    for p in range(P):
        indg[p, p // cpg] = 1.0
    inde = indg.T.copy()
    # fold the 1/cpg group-mean divisor into the indicator matmul
    return {
        "indg": indg / np.float32(cpg),
        "inde": inde,
    }


_PROGRAM_CACHE = {}


def _get_program(C, G, N, NQ, precision="bf16"):
    key = (C, G, N, NQ, precision)
    if key not in _PROGRAM_CACHE:
        _PROGRAM_CACHE[key] = build_program(C=C, G=G, N=N, NQ=NQ,
                                            precision=precision)
    return _PROGRAM_CACHE[key]


def make_in_maps(x, gn_w, gn_b, q_w, q_b, k_w, k_b, v_w, v_b, proj_w, proj_b,
                 n_cores=8, G=32):
    """Shard full inputs into per-core input maps (biases folded on host)."""
    import ml_dtypes
    BF = ml_dtypes.bfloat16
    f = lambda a: np.ascontiguousarray(np.asarray(a, dtype=np.float32))
    x = f(x)
    b, c, h, w = x.shape
    n = h * w
    qblocks = n_cores // b
    nq = n // qblocks
    cs = c // 128
    scale = np.float32(c ** -0.5)
    xf = x.reshape(b, c, n)

    def to_pcs(v):                       # [C] -> [128, CS] (c = 128*s + p)
        return np.ascontiguousarray(np.asarray(v, np.float32).reshape(cs, 128).T)

    consts = make_consts(cpg=c // G)
    wall = np.concatenate(
        [f(k_w).T, f(v_w).T, f(q_w).T * scale, f(proj_w).T], axis=1)
    cpack = np.concatenate([
        consts["indg"],
        to_pcs(gn_w), to_pcs(gn_b),
        to_pcs(f(q_b) * scale), to_pcs(k_b),
        to_pcs(f(proj_w) @ f(v_b) + f(proj_b)),
    ], axis=1)
    tail = np.concatenate([cpack.astype(np.float32).ravel(),
                           consts["inde"].ravel()])
    common = {"wall": np.ascontiguousarray(wall.astype(BF))}
    in_maps = []
    for i in range(n_cores):
        bi, qi = divmod(i, qblocks)
        xq = xf[bi][:, qi * nq:(qi + 1) * nq]
        in_maps.append({
            **common,
            "xpack": np.concatenate(
                [xf[bi].ravel(), np.ascontiguousarray(xq).ravel(), tail]),
        })
    return in_maps, (b, c, h, w, n, nq, qblocks)


def kernel(x, gn_w, gn_b, q_w, q_b, k_w, k_b, v_w, v_b, proj_w, proj_b):
    from concourse.bass_utils import run_bass_kernel_spmd

    in_maps, (b, c, h, w, n, nq, qblocks) = make_in_maps(
        x, gn_w, gn_b, q_w, q_b, k_w, k_b, v_w, v_b, proj_w, proj_b
    )
    n_cores = 8
    nc = _get_program(C=c, G=32, N=n, NQ=nq)
    res = run_bass_kernel_spmd(nc, in_maps, list(range(n_cores))).results
    out = np.empty((b, c, n), np.float32)
    for i in range(n_cores):
        bi, qi = divmod(i, qblocks)
        out[bi, :, qi * nq:(qi + 1) * nq] = res[i]["out"]
    return out.reshape(b, c, h, w)
